# revision 1
# baseline (speedup 1.0000x reference)
"""Trainium2 Bass kernel for nn_GRNNTransformGated (recursive tree GRNN over
1024 independent 10-level binary jets).

Strategy:
  - Data-parallel over jets: 8 cores x 128 trees each.
  - Canonical children layout (node k -> children 2k, 2k+1) means child
    "gathers" are stride-2 slices of the level below; the whole bottom-up
    recursion stays in SBUF (only contents in, root embeddings out).
  - Feature-major layout [128 features (partitions), nodes (free)] so every
    matmul is lhsT.T @ rhs with weight blocks stationary.
  - conv_chain collapses: for w>0, b>=0, f(f(f(x))) = w^2*relu(w*x+b) + (w*b+b).
  - sigmoid via tanh (same ACT table set as exp): r = 0.5*(tanh(q/2)+1); the
    0.5 is folded into W_h on the host.
  - softmax reciprocal on the vector engine (exp+recip can't share an ACT
    table set).
"""

import sys

for _p in ("/opt/trn_rl_repo", "/root/.axon_site/_ro/trn_rl_repo"):
    if _p not in sys.path:
        sys.path.insert(0, _p)

import numpy as np

B = 1024
L = 10
H = 128
FEAT = 7
NCORES = 8
TPC = B // NCORES          # trees per core = 128
TCH = 16                   # trees per chunk
NCHUNK = TPC // TCH        # 8 chunks
NPC = TPC * (2 ** L - 1)   # nodes per core = 130944
LOFF = [TPC * (2 ** j - 1) for j in range(L + 1)]  # level offsets in per-core ct
LEVEL_SIZES = [B * 2 ** j for j in range(L)]
OFF = np.concatenate([[0], np.cumsum(LEVEL_SIZES)]).astype(int)
INNER = LEVEL_SIZES[:-1]
COFF = np.concatenate([[0], np.cumsum(INNER)]).astype(int)

MMT = 512  # matmul node-tile size

_CACHE = {}


def _children_canonical(children):
    for j in range(L - 1):
        n = INNER[j]
        blk = children[COFF[j]:COFF[j + 1]]
        base = 2 * np.arange(n, dtype=np.int64)
        if not (np.array_equal(blk[:, 0], base) and np.array_equal(blk[:, 1], base + 1)):
            return False
    return True


def _numpy_fallback(contents, children, W_u, b_u, W_h, b_h, W_z, b_z, W_r, b_r,
                    conv_w, conv_b):
    w, b = float(conv_w[0]), float(conv_b[0])

    def conv_chain(x):
        for _ in range(3):
            x = np.maximum(w * x + b, 0.0)
        return x

    def sigmoid(x):
        return 1.0 / (1.0 + np.exp(-x))

    emb = None
    for j in reversed(range(L)):
        c = contents[OFF[j]:OFF[j + 1]]
        u = conv_chain(c @ W_u + b_u)
        if j == L - 1:
            emb = u
            continue
        ch = children[COFF[j]:COFF[j + 1]]
        h_L = emb[ch[:, 0]]
        h_R = emb[ch[:, 1]]
        hhu = np.concatenate([h_L, h_R, u], axis=1)
        r = sigmoid(hhu @ W_r + b_r)
        h_H = conv_chain((r * hhu) @ W_h + b_h)
        z = np.concatenate([h_H, hhu], axis=1) @ W_z + b_z
        zs = np.stack([z[:, :H], z[:, H:2 * H], z[:, 2 * H:3 * H], z[:, 3 * H:]], axis=-1)
        zs = zs - zs.max(axis=-1, keepdims=True)
        e = np.exp(zs)
        g = e / e.sum(axis=-1, keepdims=True)
        emb = g[..., 0] * h_H + g[..., 1] * h_L + g[..., 2] * h_R + g[..., 3] * u
    return emb.reshape(B, -1).astype(np.float32)


def _build(cw, cb, collapsible, do_affine, A, C):
    from contextlib import ExitStack

    from concourse import bacc, bass, mybir, tile

    f32 = mybir.dt.float32
    bf16 = mybir.dt.bfloat16
    AF = mybir.ActivationFunctionType
    OP = mybir.AluOpType

    nc = bacc.Bacc()

    ct_d = nc.declare_dram_parameter("ct", [FEAT, NPC], bf16, isOutput=False)
    wu_d = nc.declare_dram_parameter("wu", [FEAT, H], bf16, isOutput=False)
    wr_d = nc.declare_dram_parameter("wr", [H, 3, 3, H], bf16, isOutput=False)
    wh_d = nc.declare_dram_parameter("wh", [H, 3, H], bf16, isOutput=False)
    wz_d = nc.declare_dram_parameter("wz", [H, 4, 4, H], bf16, isOutput=False)
    bv_d = nc.declare_dram_parameter("bvec", [H, 9], f32, isOutput=False)
    id_d = nc.declare_dram_parameter("ident", [H, H], f32, isOutput=False)
    out_d = nc.declare_dram_parameter("out", [TPC, H], f32, isOutput=True)

    with ExitStack() as ctx:
        tc = ctx.enter_context(tile.TileContext(nc))
        wpool = ctx.enter_context(tc.tile_pool(name="wts", bufs=1))
        epool = ctx.enter_context(tc.tile_pool(name="emb", bufs=1))
        ctpool = ctx.enter_context(tc.tile_pool(name="ct", bufs=3))
        spool = ctx.enter_context(tc.tile_pool(name="tmp", bufs=2))
        upool = ctx.enter_context(tc.tile_pool(name="utmp", bufs=3))
        ppu = ctx.enter_context(tc.tile_pool(name="ppu", bufs=1, space="PSUM"))
        ppr = ctx.enter_context(tc.tile_pool(name="ppr", bufs=1, space="PSUM"))
        ppz = ctx.enter_context(tc.tile_pool(name="ppz", bufs=1, space="PSUM"))

        wu = wpool.tile([FEAT, H], bf16, tag="wu")
        wr = wpool.tile([H, 3, 3, H], bf16, tag="wr")
        wh = wpool.tile([H, 3, H], bf16, tag="wh")
        wz = wpool.tile([H, 4, 4, H], bf16, tag="wz")
        bv = wpool.tile([H, 9], f32, tag="bv")
        idt = wpool.tile([H, H], f32, tag="idt")
        nc.sync.dma_start(wu[:], wu_d[:])
        nc.sync.dma_start(wr[:], wr_d[:])
        nc.sync.dma_start(wh[:], wh_d[:])
        nc.sync.dma_start(wz[:], wz_d[:])
        nc.sync.dma_start(bv[:], bv_d[:])
        nc.sync.dma_start(idt[:], id_d[:])

        # emb level buffers (phase A holds one chunk; emb5 accumulates all chunks)
        e9 = epool.tile([H, TCH * 512], bf16, tag="e9")     # 8192
        e8 = epool.tile([H, TCH * 256], bf16, tag="e8")     # 4096
        e7 = epool.tile([H, TCH * 128], bf16, tag="e7")     # 2048
        e6 = epool.tile([H, TCH * 64], bf16, tag="e6")      # 1024
        emb5 = epool.tile([H, TPC * 32], bf16, tag="emb5")  # 4096 (all trees)

        def conv_emit(dst, psum_ap, bias_col):
            """dst = conv_chain(psum + b_lin) with b_lin folded in bias col."""
            nc.scalar.activation(dst, psum_ap, AF.Relu, bias=bv[:, bias_col:bias_col + 1],
                                 scale=cw)
            if collapsible:
                if do_affine:
                    nc.vector.tensor_scalar(dst, dst, A, C, OP.mult, OP.add)
            else:
                nc.scalar.activation(dst, dst, AF.Relu, bias=cb, scale=cw)
                nc.scalar.activation(dst, dst, AF.Relu, bias=cb, scale=cw)

        def inner_tile(cb3, pbase, ct_ap, out_ap, n):
            """One tile of n parent nodes at offset pbase within the level."""
            hL = cb3[:, pbase:pbase + n, 0]
            hR = cb3[:, pbase:pbase + n, 1]
            # ---- u ----
            pu = ppu.tile([H, MMT], f32, name="puh", tag="puh")
            nc.tensor.matmul(pu[:, :n], wu[:], ct_ap, start=True, stop=True)
            up = upool.tile([H, MMT], bf16, name="up", tag="up")
            u = up[:, :n]
            conv_emit(u, pu[:, :n], 0)
            rhs_k = [hL, hR, u]
            # ---- r gates (as tanh) ----
            prs = [ppr.tile([H, MMT], f32, name=f"pr{m}", tag=f"pr{m}") for m in range(3)]
            for m in range(3):
                for k in range(3):
                    nc.tensor.matmul(prs[m][:, :n], wr[:, k, m, :], rhs_k[k],
                                     start=(k == 0), stop=(k == 2))
            tts = []
            for m in range(3):
                tm = spool.tile([H, MMT], f32, name=f"t{m}", tag=f"t{m}")
                nc.scalar.activation(tm[:, :n], prs[m][:, :n], AF.Tanh,
                                     bias=bv[:, 1 + m:2 + m], scale=0.5)
                tts.append(tm)
            # ---- rh = (t+1) * hhu   (x0.5 folded into W_h) ----
            rhs_h = []
            for k in range(3):
                rk = spool.tile([H, MMT], bf16, name=f"rh{k}", tag=f"rh{k}")
                nc.vector.scalar_tensor_tensor(rk[:, :n], tts[k][:, :n], 1.0,
                                               rhs_k[k], OP.add, OP.mult)
                rhs_h.append(rk)
            ph = ppu.tile([H, MMT], f32, name="puh", tag="puh")
            for k in range(3):
                nc.tensor.matmul(ph[:, :n], wh[:, k, :], rhs_h[k][:, :n],
                                 start=(k == 0), stop=(k == 2))
            hp = upool.tile([H, MMT], bf16, name="hp", tag="hp")
            hH = hp[:, :n]
            conv_emit(hH, ph[:, :n], 4)
            # ---- z ----
            zk = [hH, hL, hR, u]
            pzs = [ppz.tile([H, MMT], f32, name=f"pz{m}", tag=f"pz{m}") for m in range(4)]
            for m in range(4):
                for k in range(4):
                    nc.tensor.matmul(pzs[m][:, :n], wz[:, k, m, :], zk[k],
                                     start=(k == 0), stop=(k == 3))
            es = []
            for m in range(4):
                em = spool.tile([H, MMT], f32, name=f"e{m}", tag=f"e{m}")
                nc.scalar.activation(em[:, :n], pzs[m][:, :n], AF.Exp,
                                     bias=bv[:, 5 + m:6 + m])
                es.append(em)
            e0, e1, e2, e3 = [e[:, :n] for e in es]
            # ---- softmax-weighted combine ----
            s01 = spool.tile([H, MMT], f32, name="s01", tag="s01")
            s23 = spool.tile([H, MMT], f32, name="s23", tag="s23")
            nc.vector.tensor_tensor(s01[:, :n], e0, e1, OP.add)
            nc.gpsimd.tensor_tensor(s23[:, :n], e2, e3, OP.add)
            nc.vector.tensor_tensor(s01[:, :n], s01[:, :n], s23[:, :n], OP.add)
            rcp = spool.tile([H, MMT], f32, name="rcp", tag="rcp")
            nc.vector.reciprocal_approx_fast(rcp[:, :n], s01[:, :n])
            nc.vector.tensor_tensor(e0, e0, hH, OP.mult)
            nc.gpsimd.tensor_tensor(e1, e1, hL, OP.mult)
            nc.gpsimd.tensor_tensor(e2, e2, hR, OP.mult)
            nc.gpsimd.tensor_tensor(e3, e3, u, OP.mult)
            nc.vector.tensor_tensor(e0, e0, e1, OP.add)
            nc.vector.tensor_tensor(e2, e2, e3, OP.add)
            nc.vector.tensor_tensor(e0, e0, e2, OP.add)
            nc.vector.tensor_tensor(out_ap, e0, rcp[:, :n], OP.mult)

        def run_level(nj, ct_base, cbuf, obuf_ap):
            """One level with nj parents; children in cbuf (2*nj wide)."""
            cb3 = cbuf[:].rearrange("p (n two) -> p n two", two=2)
            done = 0
            while done < nj:
                piece = min(2048, nj - done)
                ctt = ctpool.tile([FEAT, 2048], bf16, name="ctt", tag="ctt")
                nc.sync.dma_start(ctt[:, :piece],
                                  ct_d[:, ct_base + done:ct_base + done + piece])
                for s in range(0, piece, MMT):
                    n = min(MMT, piece - s)
                    pbase = done + s
                    inner_tile(cb3, pbase, ctt[:, s:s + n],
                               obuf_ap[:, pbase:pbase + n], n)
                done += piece

        # ================= phase A: per-chunk levels 9..5 =================
        for c in range(NCHUNK):
            # leaf level 9
            nleaf = TCH * 512  # 8192
            base9 = LOFF[9] + c * nleaf
            for hpiece in range(0, nleaf, 2048):
                ctt = ctpool.tile([FEAT, 2048], bf16, name="ctt", tag="ctt")
                nc.sync.dma_start(ctt[:], ct_d[:, base9 + hpiece:base9 + hpiece + 2048])
                for s in range(0, 2048, MMT):
                    pu = ppu.tile([H, MMT], f32, name="puh", tag="puh")
                    nc.tensor.matmul(pu[:], wu[:], ctt[:, s:s + MMT],
                                     start=True, stop=True)
                    dst = e9[:, hpiece + s:hpiece + s + MMT]
                    nc.scalar.activation(dst, pu[:], AF.Relu,
                                         bias=bv[:, 0:1], scale=cw)
                    if not collapsible:
                        nc.scalar.activation(dst, dst, AF.Relu, bias=cb, scale=cw)
                        nc.scalar.activation(dst, dst, AF.Relu, bias=cb, scale=cw)
                if collapsible and do_affine:
                    big = e9[:, hpiece:hpiece + 2048]
                    nc.vector.tensor_scalar(big, big, A, C, OP.mult, OP.add)
            # inner levels 8..5
            for j, (cbuf, obuf) in zip(
                    range(8, 4, -1),
                    [(e9, e8), (e8, e7), (e7, e6), (e6, None)]):
                nj = TCH * (2 ** j)
                if j == 5:
                    ob = emb5[:, c * 512:(c + 1) * 512]
                else:
                    ob = obuf[:, :nj]
                run_level(nj, LOFF[j] + c * nj, cbuf, ob)

        # ================= phase B: levels 4..0, all trees =================
        # reuse dead phase-A buffers for the tail levels
        e4 = e8[:, :2048]
        e3 = e7[:, :1024]
        e2 = e6[:, :512]
        e1 = e8[:, 2048:2048 + 256]
        e0f = epool.tile([H, TPC], f32, tag="e0f")
        e0 = e0f[:, :TPC]
        chain = [(emb5[:], e4), (e4, e3), (e3, e2), (e2, e1), (e1, e0)]
        for j, (cbap, ob) in zip(range(4, -1, -1), chain):
            nj = TPC * (2 ** j)
            cb3v = cbap.rearrange("p (n two) -> p n two", two=2)
            done = 0
            while done < nj:
                piece = min(2048, nj - done)
                ctt = ctpool.tile([FEAT, 2048], bf16, name="ctt", tag="ctt")
                nc.sync.dma_start(ctt[:, :piece],
                                  ct_d[:, LOFF[j] + done:LOFF[j] + done + piece])
                for s in range(0, piece, MMT):
                    n = min(MMT, piece - s)
                    pbase = done + s
                    inner_tile(cb3v, pbase, ctt[:, s:s + n], ob[:, pbase:pbase + n], n)
                done += piece

        # ================= output transpose + store =================
        pt = ppz.tile([H, H], f32, name="pz0", tag="pz0")
        nc.tensor.matmul(pt[:], e0, idt[:], is_transpose=True, start=True, stop=True)
        osb = spool.tile([H, H], f32, name="osb", tag="osb")
        nc.vector.tensor_copy(osb[:], pt[:])
        nc.sync.dma_start(out_d[:], osb[:])

    nc.compile()
    if not nc.is_finalized():
        nc.finalize()
    return nc


def _prepare(inputs):
    contents = np.ascontiguousarray(np.asarray(inputs["contents"], np.float32))
    W_u = np.asarray(inputs["W_u"], np.float32)
    b_u = np.asarray(inputs["b_u"], np.float32)
    W_h = np.asarray(inputs["W_h"], np.float32)
    b_h = np.asarray(inputs["b_h"], np.float32)
    W_z = np.asarray(inputs["W_z"], np.float32)
    b_z = np.asarray(inputs["b_z"], np.float32)
    W_r = np.asarray(inputs["W_r"], np.float32)
    b_r = np.asarray(inputs["b_r"], np.float32)
    cw = float(np.asarray(inputs["conv_w"]).reshape(-1)[0])
    cb = float(np.asarray(inputs["conv_b"]).reshape(-1)[0])

    # per-core feature-major contents, level-major columns
    cts = np.empty((NCORES, FEAT, NPC), np.float32)
    col = 0
    for j in range(L):
        n = TPC * 2 ** j
        blk = contents[OFF[j]:OFF[j + 1]].reshape(NCORES, n, FEAT)
        cts[:, :, col:col + n] = blk.transpose(0, 2, 1)
        col += n

    wr_np = np.ascontiguousarray(W_r.reshape(3, H, 3, H).transpose(1, 0, 2, 3))
    wz_np = np.ascontiguousarray(W_z.reshape(4, H, 4, H).transpose(1, 0, 2, 3))
    wh_np = np.ascontiguousarray((0.5 * W_h).reshape(3, H, H).transpose(1, 0, 2))

    bvec = np.zeros((H, 9), np.float32)
    bvec[:, 0] = cw * b_u + cb
    bvec[:, 1:4] = 0.5 * b_r.reshape(3, H).T
    bvec[:, 4] = cw * b_h + cb
    bvec[:, 5:9] = b_z.reshape(4, H).T

    import ml_dtypes

    bf = ml_dtypes.bfloat16
    common = {
        "wu": np.ascontiguousarray(W_u).astype(bf),
        "wr": wr_np.astype(bf), "wh": wh_np.astype(bf), "wz": wz_np.astype(bf),
        "bvec": bvec,
        "ident": np.eye(H, dtype=np.float32),
    }
    in_maps = [dict(common, ct=np.ascontiguousarray(cts[c]).astype(bf))
               for c in range(NCORES)]
    return in_maps, cw, cb


def kernel(**inputs):
    children = np.asarray(inputs["children"])
    cw = float(np.asarray(inputs["conv_w"]).reshape(-1)[0])
    cb = float(np.asarray(inputs["conv_b"]).reshape(-1)[0])
    collapsible = (cw >= 0.0) and (cb >= 0.0)
    if not _children_canonical(children):
        args = {k: np.asarray(v) for k, v in inputs.items()}
        return _numpy_fallback(**args)

    from concourse.bass_utils import run_bass_kernel_spmd

    A = cw * cw
    C = cw * cb + cb
    do_affine = not (A == 1.0 and C == 0.0)

    key = (cw, cb, collapsible, do_affine)
    if key not in _CACHE:
        _CACHE[key] = _build(cw, cb, collapsible, do_affine, A, C)
    nc = _CACHE[key]

    in_maps, _, _ = _prepare(inputs)
    res = run_bass_kernel_spmd(nc, in_maps, list(range(NCORES)))
    outs = [res.results[c]["out"] for c in range(NCORES)]
    return np.ascontiguousarray(np.concatenate(outs, axis=0).astype(np.float32))


if __name__ == "__main__":
    rng = np.random.default_rng(0)
    print("kernel module loaded")

